# revision 8
# baseline (speedup 1.0000x reference)
"""Trainium2 Bass kernel for nn_ActorNetSpiking — v9 (promoted v8).

Data-parallel over 8 NeuronCores: batch 4096 -> 512 per core.

Changes vs v6 (1513866 ns baseline):
- Weights: fp16 halves (1 or 2) instead of bf16 x3. PE matmul cost is
  keyed on the moving (rhs) dtype; fp16 lhsT x bf16 rhs measured exact
  on HW. 3x (or 1.5x) fewer chunk matmuls.
- LIF elementwise: two custom fused DVE ops replace the v/spike/vt
  passes for DVE-class layers (v never materialized):
    LIF_HIST: hist = ((vt + u) <= 0.5) * 2^t
    LIF_VT:   vt'  = select((vt + u) <= 0.5, (vt + u) * 0.75, 0)
  vt is stored UNSCALED (0.75 * v * ns), so no cross-step scale
  juggling and no t==0 special case (vt memset 0 once).
- Pool-class layers (POOL_LAYERS) run the same math as std 3-op
  sequences on the otherwise-idle GpSimd engine:
    v = (vt mult 1.0) add u; hist = (v is_le .5) * 2^t;
    vt' = (v * 0.75*2^-t) * hist
- u-chain in PSUM, ACT drains with beta, fp32 identity carry: as v6.
"""

import os
import numpy as np
import ml_dtypes

import concourse.bass as bass
import concourse.bacc as bacc_mod
import concourse.tile as tile
from concourse import mybir
from concourse._compat import with_exitstack
from concourse.bass_utils import run_bass_kernel_spmd

import concourse.dve_ops as dve_ops_mod
from concourse.dve_spec import Spec, Src0, Src1, C0, C1, select, Zero, lower
from concourse.dve_uop import DveOpSpec
from concourse.dve_table_gen import dve_ver_for

F32 = mybir.dt.float32
BF16 = mybir.dt.bfloat16
F16 = mybir.dt.float16
AF = mybir.ActivationFunctionType
OP = mybir.AluOpType

N_CORES = 8
B_FULL = 4096
B = B_FULL // N_CORES  # 512 per core
T = 50
Tb = 4  # DMA/hist ring block (hist t-ring depth)
Tc = 4  # PSUM chain length; MUST equal Tb (bank-sharing + hist ring)
WMODE = 'bf16x3'
N_HALVES = 3
LAYER_HALVES = {0: 3, 1: 3, 2: 2, 3: 2, 4: 2, 5: 2, 6: 2}
W_NPDT = 'bf16'
# number of trailing tiles per layer whose LIF runs as std 3-op sequences on
# GpSimd (Pool); leading tiles use the fused custom DVE ops
POOL_TILES = {0: 0, 1: 0, 2: 0, 3: 0, 4: 0, 5: 0}  # GpSimd lacks STT/PSUM
STD_LIF = False

CONV = [  # (Lin, Lout, Cin, Cout)
    (360, 178, 1, 5),
    (178, 87, 5, 5),
    (87, 42, 5, 5),
]

# hist slot base per layer-INPUT (layers 1..6 read hist; layer 0 reads scan)
HIST_BASE = [None, 0, 7, 11, 13, 15, 17]
N_SLOTS = 18
# PSUM bank map per layer (bank index list)
BANKS = [
    [0, 1, 2, 3, 4, 5, 6],
    [7, 0, 1, 2],
    [3, 4],
    [5, 6],
    [7, 0],
    [1],
    [2],
]


def _register_dve_op(name, spec, subdim=False):
    for op in dve_ops_mod.OPS:
        if op.name == name:
            return op
    row = dve_ops_mod._CUSTOM_DVE_ROW_BASE + len(dve_ops_mod.OPS)
    assert row < 0x20, "custom DVE row overflow"
    dve_ops_mod._SUB_OPCODE_FOR_NAME[name] = row
    rd1 = dve_ops_mod.has_src1(spec)
    shas = {}
    for ver in ("v3", "v4"):
        try:
            s = DveOpSpec(name=name, opcode=row, uops=lower(spec, ver=ver),
                          rd1_en=rd1)
            shas[ver] = s.sha(ver)
        except Exception:
            pass
    op = dve_ops_mod.DveOp(name, spec, subdim=subdim, uops_sha=shas)
    dve_ops_mod.OPS.append(op)
    return op


# out = ((vt + u) <= 0.5) * s1     [s0 = threshold, s1 = hist scale 2^t]
LIF_HIST = _register_dve_op(
    "LIF_HIST_SNN",
    Spec(body=((Src0 + Src1) <= C0) * C1,
         reference=lambda in0, in1, s0, s1: ((in0 + in1) <= s0) * s1))
# out = select((vt + u) <= 0.5, (vt + u) * s1, 0)   [s1 = 0.75]
LIF_VT = _register_dve_op(
    "LIF_VT_SNN",
    Spec(body=select((Src0 + Src1) <= C0, (Src0 + Src1) * C1, Zero),
         reference=lambda in0, in1, s0, s1: np.where(
             (in0 + in1) <= s0, (in0 + in1) * s1, 0.0)))


def _build_banded(w, b, Lin, Lout, Cin, Cout):
    rows_in, rows_out = Lin * Cin, Lout * Cout
    Wd = np.zeros((rows_in, rows_out), np.float32)
    K = w.shape[2]
    for l in range(Lout):
        for k in range(K):
            li = 2 * l + k
            Wd[li * Cin:(li + 1) * Cin, l * Cout:(l + 1) * Cout] = w[:, :, k].T
    bias = np.tile(b, Lout)
    return Wd, bias


def _plan_layers(inp, n_halves):
    """Per layer: tiles with 128-grid-aligned chunks, bias consts.

    Weights stored as `n_halves` fp16 splits; effective weight =
    sum of halves (exact fp16 values).
    """
    wdt = np.float16 if W_NPDT == 'f16' else ml_dtypes.bfloat16
    mats = []
    for i, (Lin, Lout, Cin, Cout) in enumerate(CONV):
        w, b = inp[f'conv{i+1}_w'], inp[f'conv{i+1}_b']
        mats.append(_build_banded(w, b, Lin, Lout, Cin, Cout))
    fw, fb = inp['fc1_w'], inp['fc1_b']
    Wd = np.zeros((216, 256), np.float32)
    for j in range(210):
        l3, co = j // 5, j % 5
        Wd[j, :] = fw[:, co * 42 + l3]
    Wd[210:216, :] = fw[:, 210:216].T
    mats.append((Wd, fb.copy()))
    for i in (2, 3, 4):
        fw, fb = inp[f'fc{i}_w'], inp[f'fc{i}_b']
        mats.append((fw.T.astype(np.float32), fb.copy()))

    layers = []
    for lidx, (Wd, bias) in enumerate(mats):
        rows_in, rows_out = Wd.shape
        ns_rows = np.zeros(rows_in, bool)
        if lidx >= 1:
            ns_rows[:] = True
            if lidx == 3:
                ns_rows[210:216] = False
        Ws = Wd.copy()
        Ws[ns_rows, :] *= -1.0  # stored weight: -W on ns rows
        halves = []
        rem = Ws.astype(np.float64)
        for _ in range(LAYER_HALVES.get(lidx, n_halves)):
            h = rem.astype(np.float32).astype(wdt)
            halves.append(h)
            rem = rem - h.astype(np.float64)
        Weff = np.zeros_like(Ws, np.float64)
        for h in halves:
            Weff += h.astype(np.float64)
        # rowsum of EFFECTIVE stored weights over ns rows, negated back:
        # syn_true = stored @ ns_enc + rowsum ; rowsum = sum_ns(-Weff)
        rowsum = (-Weff * ns_rows[:, None]).sum(axis=0)
        c = bias.astype(np.float64) + rowsum
        tiles = []
        for m0 in range(0, rows_out, 128):
            m1 = min(m0 + 128, rows_out)
            sub = Weff[:, m0:m1]
            nz = np.nonzero(np.any(sub != 0.0, axis=1))[0]
            k0, k1 = int(nz.min()), int(nz.max()) + 1
            chunks = []
            for g in range(k0 // 128, (k1 + 127) // 128):
                a = g * 128
                bnd = min(a + 128, k1, rows_in)
                chunks.append((a, bnd,
                               [np.asarray(h[a:bnd, m0:m1]) for h in halves]))
            tiles.append(dict(m0=m0, m1=m1, chunks=chunks, c=c[m0:m1]))
        layers.append(dict(rows_in=rows_in, rows_out=rows_out, tiles=tiles,
                           G=len(tiles), n_halves=n_halves))
    return layers


def _pack_weights(layers):
    """Pack all chunk halves into one [128, total] fp16 array + beta table."""
    total = 0
    for L in layers:
        for tl in L['tiles']:
            for (a, b_, hs) in tl['chunks']:
                total += hs[0].shape[1] * len(hs)
    wpack = np.zeros((128, total), np.float32)
    off = 0
    for L in layers:
        for tl in L['tiles']:
            tl['offs'] = []
            for (a, b_, hs) in tl['chunks']:
                K, M = hs[0].shape
                hoffs = []
                for h in hs:
                    wpack[:K, off:off + M] = h.astype(np.float32)
                    hoffs.append(off)
                    off += M
                tl['offs'].append(hoffs)
    # beta: per tile column per local tau: c * (2 - 2^-tau)
    ntiles = sum(L['G'] for L in layers)
    btab = np.zeros((128, ntiles * Tc), np.float32)
    ti = 0
    for L in layers:
        for tl in L['tiles']:
            tl['bidx'] = ti
            g = 2.0 - np.power(2.0, -np.arange(Tc, dtype=np.float64))
            btab[:tl['m1'] - tl['m0'], ti * Tc:(ti + 1) * Tc] = (
                tl['c'][:, None] * g[None, :]).astype(np.float32)
            ti += 1
    wdt = np.float16 if W_NPDT == 'f16' else ml_dtypes.bfloat16
    return wpack.astype(wdt), btab


@with_exitstack
def _emit(ctx, tc, layers, wcols, nbt, prm):
    nc = tc.nc
    persist = ctx.enter_context(tc.tile_pool(name="persist", bufs=1))
    scanp = ctx.enter_context(tc.tile_pool(name="scanin", bufs=2))
    psump = ctx.enter_context(tc.tile_pool(name="psum", bufs=1, space="PSUM"))

    WSB_DT = F16 if W_NPDT == 'f16' else BF16
    wsb = persist.tile([128, wcols], WSB_DT, tag="wsb")
    t0_cols = sum(hs[0].shape[1] * len(hs)
                  for (a, b_, hs) in layers[0]['tiles'][0]['chunks'])
    c1_cols = sum(hs[0].shape[1] * len(hs) for tl in layers[0]['tiles']
                  for (a, b_, hs) in tl['chunks'])
    nc.sync.dma_start(wsb[:, :t0_cols], prm['w'][:, :t0_cols])
    bsb = persist.tile([128, nbt], F32, tag="bsb")
    nc.sync.dma_start(bsb[:], prm['bias'][:])
    ident = persist.tile([128, 128], F32, tag="ident")
    nc.sync.dma_start(ident[:], prm['ident'][:])

    hist = persist.tile([128, N_SLOTS, Tb, B], BF16, tag="hist")

    # per-layer contiguous state slices
    goffs = []
    tot = 0
    for L in layers:
        goffs.append(tot)
        tot += L['G']
    usb_all = persist.tile([128, tot * B], F32, tag="usb")
    vtb_all = persist.tile([128, tot * B], F32, tag="vtb")
    usb = [usb_all[:, goffs[i] * B:(goffs[i] + L['G']) * B]
           for i, L in enumerate(layers)]
    vtb = [vtb_all[:, goffs[i] * B:(goffs[i] + L['G']) * B]
           for i, L in enumerate(layers)]
    acc = persist.tile([2, B], F32, tag="acc")
    ns4 = persist.tile([2, B], F32, tag="ns4")

    psum = psump.tile([128, 8 * 512], F32, tag="psum")

    # scratch v tiles for Pool-handled tile ranges
    vscratch = {}
    for li, np_ in POOL_TILES.items():
        if np_ > 0:
            vscr_tile = persist.tile([128, np_ * B], F32,
                                     name=f"vscr{li}", tag=f"vscr{li}")
            vscratch[li] = vscr_tile

    # usb rows beyond each tile's M are read by full-width LIF ops (stay 0:
    # drains write [:M] only). vtb starts 0 (v_0 = u_0). hist slot 12 rows
    # 88.. are read by fc1 chunks and never written.
    nc.vector.memset(usb_all[:], 0.0)
    nc.vector.memset(vtb_all[:], 0.0)
    nc.vector.memset(acc[:], 0.0)
    nc.vector.memset(hist[:, 12], 0.0)

    n_c1 = 3
    nblocks = (T + Tb - 1) // Tb
    for blk in range(nblocks):
        t0 = blk * Tb
        tb = min(Tb, T - t0)
        sc = scanp.tile([128, n_c1, Tb, B], BF16, tag="scan")
        nc.sync.dma_start(sc[:, :, :tb, :], prm['scan2'][:, :, t0:t0 + tb, :])
        if blk == 0:
            nc.sync.dma_start(wsb[:, t0_cols:c1_cols],
                              prm['w'][:, t0_cols:c1_cols])
            nc.sync.dma_start(wsb[:, c1_cols:], prm['w'][:, c1_cols:])
        nc.sync.dma_start(hist[82:88, 12, :tb, :],
                          prm['normal'][:, t0:t0 + tb, :])

        def emit_pkg(li, t, blk=blk, t0=t0, tb=tb, sc=sc):
            L = layers[li]
            G = L['G']
            tiles = L['tiles']
            u_l, vt_l = usb[li], vtb[li]
            s0 = HIST_BASE[li + 1] if li < 6 else None
            t_abs = t0 + t
            tau = t_abs % Tc           # position in the PSUM chain
            banks = [(b + 2 * (t_abs // Tc)) % 8 for b in BANKS[li]]
            sc2 = float(2.0 ** tau)      # rhs/hist scale this step
            sc2m = float(2.0 ** (-tau))  # drain scale

            # --- PE: all chunk matmuls (+carry) of the package, dense ---
            for ti_, tl in enumerate(tiles):
                M = tl['m1'] - tl['m0']
                bk = banks[ti_]
                out_ap = psum[:M, bk * 512:bk * 512 + B]
                first_mm = (tau == 0 and t_abs == 0)
                if tau == 0 and t_abs > 0:
                    # cross-block carry on ACT: bank := Id(0.5*u_prev)
                    # (overwrite; exact). Chunks then accumulate with
                    # start=False on top -- engine-write + matmul-accumulate
                    # mixing is the v6-validated dve_carry pattern.
                    nc.scalar.activation(
                        out_ap, u_l[:M, ti_ * B:(ti_ + 1) * B],
                        AF.Identity, scale=0.5)
                nch = len(tl['chunks'])
                for ci_, ((a, b_, hs), hoffs) in enumerate(
                        zip(tl['chunks'], tl['offs'])):
                    K = b_ - a
                    g_src = a // 128
                    if li == 0:
                        rhs = sc[0:K, g_src, t, :]
                    else:
                        rhs = hist[0:K, HIST_BASE[li] + g_src, t, :]
                    for hi_ in range(len(hs)):
                        st = first_mm and ci_ == 0 and hi_ == 0
                        nc.tensor.matmul(
                            out_ap, wsb[0:K, hoffs[hi_]:hoffs[hi_] + M],
                            rhs,
                            start=st,
                            stop=((tau == Tc - 1 or t_abs == T - 1)
                                  and ci_ == nch - 1
                                  and hi_ == len(hs) - 1),
                            skip_group_check=True)
            # --- ACT: per-tile drains u_true = Id(2^-tau * U + beta) ---
            for ti_, tl in enumerate(tiles):
                M = tl['m1'] - tl['m0']
                bk = banks[ti_]
                out_ap = psum[:M, bk * 512:bk * 512 + B]
                col = tl['bidx'] * Tc + tau
                nc.scalar.activation(
                    u_l[:M, ti_ * B:(ti_ + 1) * B], out_ap,
                    AF.Identity, bias=bsb[:M, col:col + 1], scale=sc2m)

            # ---- LIF: hist = ((vt+u) <= .5)*2^t ; vt' = sel(v<=.5, .75v, 0)
            if li == 6:
                nc.vector._custom_dve(LIF_HIST, out=ns4[:],
                                      in0=vt_l[:2, :], in1=u_l[:2, :],
                                      s0=0.5, s1=1.0)
                nc.vector._custom_dve(LIF_VT, out=vt_l[:2, :],
                                      in0=vt_l[:2, :], in1=u_l[:2, :],
                                      s0=0.5, s1=0.75)
                # acc += ns4 - 1  (negated spike count)
                nc.vector.scalar_tensor_tensor(
                    acc[:], ns4[:], 1.0, acc[:],
                    op0=OP.subtract, op1=OP.add)
                return

            npool = POOL_TILES[li]
            kd = G - npool  # leading tiles on DVE (fused custom ops)
            # DVE tiles: per-tile fused ops (short dependency chains)
            for ti_ in range(kd):
                tl = tiles[ti_]
                M = tl['m1'] - tl['m0']
                sl = slice(ti_ * B, (ti_ + 1) * B)
                if STD_LIF:
                    # v6-style std ops: v overwrites vt in place, then vt
                    # is recomputed from v and hist
                    nc.vector.scalar_tensor_tensor(
                        vt_l[:M, sl], vt_l[:M, sl], 1.0, u_l[:M, sl],
                        op0=OP.mult, op1=OP.add)
                    nc.vector.tensor_scalar(
                        hist[:M, s0 + ti_, t, :], vt_l[:M, sl],
                        0.5, sc2, op0=OP.is_le, op1=OP.mult)
                    nc.vector.scalar_tensor_tensor(
                        vt_l[:M, sl], vt_l[:M, sl], 0.75 * sc2m,
                        hist[:M, s0 + ti_, t, :], op0=OP.mult, op1=OP.mult)
                    continue
                nc.vector._custom_dve(
                    LIF_HIST, out=hist[:M, s0 + ti_, t, :],
                    in0=vt_l[:M, sl], in1=u_l[:M, sl], s0=0.5, s1=sc2)
                nc.vector._custom_dve(
                    LIF_VT, out=vt_l[:M, sl],
                    in0=vt_l[:M, sl], in1=u_l[:M, sl], s0=0.5, s1=0.75)
            # Pool tiles: std 3-op sequence on the contiguous trailing slice
            if npool > 0:
                vscr = vscratch[li]
                psl = slice(kd * B, G * B)
                if li == 2:
                    # conv3 tail tile M=82: write hist [:M] only (slot 12
                    # rows 82.. hold normal spikes / zeros)
                    M = tiles[kd]['m1'] - tiles[kd]['m0']
                    assert npool == 1
                    nc.gpsimd.scalar_tensor_tensor(
                        vscr[:M, :], vt_l[:M, psl], 1.0, u_l[:M, psl],
                        op0=OP.mult, op1=OP.add)
                    nc.gpsimd.tensor_scalar(
                        hist[:M, s0 + kd, t, :], vscr[:M, :512],
                        0.5, sc2, op0=OP.is_le, op1=OP.mult)
                    nc.gpsimd.scalar_tensor_tensor(
                        vt_l[:M, psl], vscr[:M, :512],
                        0.75 * (2.0 ** (-tau)), hist[:M, s0 + kd, t, :],
                        op0=OP.mult, op1=OP.mult)
                else:
                    h3d = hist[:, s0 + kd:s0 + G, t, :]
                    v3d = vscr.rearrange("p (g b) -> p g b", b=B)
                    nc.gpsimd.scalar_tensor_tensor(
                        vscr[:], vt_l[:, psl], 1.0, u_l[:, psl],
                        op0=OP.mult, op1=OP.add)
                    nc.gpsimd.tensor_scalar(
                        h3d, v3d, 0.5, sc2, op0=OP.is_le, op1=OP.mult)
                    nc.gpsimd.scalar_tensor_tensor(
                        vt_l.rearrange("p (g b) -> p g b", b=B)[:, kd:G, :],
                        v3d, 0.75 * (2.0 ** (-tau)), h3d,
                        op0=OP.mult, op1=OP.mult)

        # wavefront emission: conv1 first, then layers 1..6 skewed by 2
        for t in range(tb):
            emit_pkg(0, t)
        rest = sorted((2 * (li - 1) + t, -li, li, t)
                      for li in range(1, 7) for t in range(tb))
        for _, _, li, t in rest:
            emit_pkg(li, t)

    out_sb = persist.tile([2, B], F32, tag="outsb")
    nc.vector.tensor_scalar_mul(out_sb[:], acc[:], -1.0 / T)
    nc.sync.dma_start(prm['out'][:], out_sb[:])



def build_nc(layers, wcols, nbt):
    nc = bacc_mod.Bacc()
    prm = dict(
        scan2=nc.declare_dram_parameter("scan2", [128, 3 * T * B], BF16,
                                        isOutput=False).rearrange(
                                            "p (s t b) -> p s t b", t=T, b=B),
        normal=nc.declare_dram_parameter("normal", [6, T * B], BF16,
                                         isOutput=False).rearrange(
                                             "l (t b) -> l t b", b=B),
        w=nc.declare_dram_parameter("w", [128, wcols],
                                    F16 if W_NPDT == 'f16' else BF16,
                                    isOutput=False),
        bias=nc.declare_dram_parameter("bias", [128, nbt], F32, isOutput=False),
        ident=nc.declare_dram_parameter("ident", [128, 128], F32,
                                        isOutput=False),
        out=nc.declare_dram_parameter("out", [2, B], F32, isOutput=True),
    )

    with tile.TileContext(nc) as tc:
        _emit(tc, layers, wcols, nbt, prm)
    nc.compile()
    return nc


_NC_CACHE = {}


def kernel(normal_spikes, scan_spikes, batch_size,
           conv1_w, conv1_b, conv2_w, conv2_b, conv3_w, conv3_b,
           fc1_w, fc1_b, fc2_w, fc2_b, fc3_w, fc3_b, fc4_w, fc4_b):
    inp = dict(conv1_w=conv1_w, conv1_b=conv1_b, conv2_w=conv2_w,
               conv2_b=conv2_b, conv3_w=conv3_w, conv3_b=conv3_b,
               fc1_w=fc1_w, fc1_b=fc1_b, fc2_w=fc2_w, fc2_b=fc2_b,
               fc3_w=fc3_w, fc3_b=fc3_b, fc4_w=fc4_w, fc4_b=fc4_b)
    inp = {k: np.asarray(v, np.float32) for k, v in inp.items()}
    layers = _plan_layers(inp, N_HALVES)
    wpack, btab = _pack_weights(layers)
    wcols, nbt = wpack.shape[1], btab.shape[1]

    key = (wcols, nbt, WMODE)
    if key not in _NC_CACHE:
        _NC_CACHE[key] = build_nc(layers, wcols, nbt)
    nc = _NC_CACHE[key]
    kernel._last_nc = nc

    bf = ml_dtypes.bfloat16
    # host prep: time-major feature-major + 2^tau pre-scale (exact in bf16)
    scales = (2.0 ** (np.arange(T) % Tc)).astype(np.float32)  # [T]
    scan_t = np.asarray(scan_spikes)[:, 0].transpose(1, 2, 0)
    scan_t = (scan_t * scales[None, :, None]).astype(bf)
    norm_t = np.asarray(normal_spikes).transpose(1, 2, 0)
    norm_t = (norm_t * scales[None, :, None]).astype(bf)
    ident = np.eye(128, dtype=np.float32) * 0.5
    n_c1 = 3
    scan_rep = np.zeros((128, n_c1, T, B_FULL), bf)
    for g in range(3):
        p = min(128, 360 - g * 128)
        scan_rep[:p, g] = scan_t[g * 128:g * 128 + p]

    in_maps = []
    for c in range(N_CORES):
        sl = slice(c * B, (c + 1) * B)
        in_maps.append(dict(
            scan2=np.ascontiguousarray(
                scan_rep[:, :, :, sl]).reshape(128, n_c1 * T * B),
            normal=np.ascontiguousarray(norm_t[:, :, sl]).reshape(6, T * B),
            w=wpack, bias=btab, ident=ident))
    import time as _time
    t0 = _time.time()
    try:
        res = run_bass_kernel_spmd(nc, in_maps, list(range(N_CORES)))
    except ModuleNotFoundError:
        os.environ["BASS_NEVER_TRACE"] = "1"
        res = run_bass_kernel_spmd(nc, in_maps, list(range(N_CORES)))
    wall1 = _time.time() - t0
    outs = [res.results[c]["out"] for c in range(N_CORES)]

    full = np.concatenate([o.T for o in outs], axis=0).astype(np.float32)
    kernel._last_exec_ns = res.exec_time_ns
    kernel._wall_exec_s = wall1
    it = getattr(res, 'instructions_and_trace', None)
    kernel._last_trace = it[1] if it else None
    return full


# revision 9
# speedup vs baseline: 1.0242x; 1.0242x over previous
"""Trainium2 Bass kernel for nn_ActorNetSpiking — v9 (promoted v8).

Data-parallel over 8 NeuronCores: batch 4096 -> 512 per core.

Changes vs v6 (1513866 ns baseline):
- Weights: fp16 halves (1 or 2) instead of bf16 x3. PE matmul cost is
  keyed on the moving (rhs) dtype; fp16 lhsT x bf16 rhs measured exact
  on HW. 3x (or 1.5x) fewer chunk matmuls.
- LIF elementwise: two custom fused DVE ops replace the v/spike/vt
  passes for DVE-class layers (v never materialized):
    LIF_HIST: hist = ((vt + u) <= 0.5) * 2^t
    LIF_VT:   vt'  = select((vt + u) <= 0.5, (vt + u) * 0.75, 0)
  vt is stored UNSCALED (0.75 * v * ns), so no cross-step scale
  juggling and no t==0 special case (vt memset 0 once).
- Pool-class layers (POOL_LAYERS) run the same math as std 3-op
  sequences on the otherwise-idle GpSimd engine:
    v = (vt mult 1.0) add u; hist = (v is_le .5) * 2^t;
    vt' = (v * 0.75*2^-t) * hist
- u-chain in PSUM, ACT drains with beta, fp32 identity carry: as v6.
"""

import os
import numpy as np
import ml_dtypes

import concourse.bass as bass
import concourse.bacc as bacc_mod
import concourse.tile as tile
from concourse import mybir
from concourse._compat import with_exitstack
from concourse.bass_utils import run_bass_kernel_spmd

import concourse.dve_ops as dve_ops_mod
from concourse.dve_spec import Spec, Src0, Src1, C0, C1, select, Zero, lower
from concourse.dve_uop import DveOpSpec
from concourse.dve_table_gen import dve_ver_for

F32 = mybir.dt.float32
BF16 = mybir.dt.bfloat16
F16 = mybir.dt.float16
AF = mybir.ActivationFunctionType
OP = mybir.AluOpType

N_CORES = 8
B_FULL = 4096
B = B_FULL // N_CORES  # 512 per core
T = 50
Tb = 4  # DMA/hist ring block (hist t-ring depth)
Tc = 4  # PSUM chain length; MUST equal Tb (bank-sharing + hist ring)
WMODE = 'bf16x3'
N_HALVES = 3
LAYER_HALVES = {0: 3, 1: 3, 2: 2, 3: 2, 4: 2, 5: 2, 6: 2}
W_NPDT = 'bf16'
# number of trailing tiles per layer whose LIF runs as std 3-op sequences on
# GpSimd (Pool); leading tiles use the fused custom DVE ops
POOL_TILES = {0: 0, 1: 0, 2: 0, 3: 0, 4: 0, 5: 0}  # GpSimd lacks STT/PSUM
STD_LIF = False

CONV = [  # (Lin, Lout, Cin, Cout)
    (360, 178, 1, 5),
    (178, 87, 5, 5),
    (87, 42, 5, 5),
]

# hist slot base per layer-INPUT (layers 1..6 read hist; layer 0 reads scan)
HIST_BASE = [None, 0, 7, 11, 13, 15, 17]
N_SLOTS = 18
# PSUM bank map per layer (bank index list)
BANKS = [
    [0, 1, 2, 3, 4, 5, 6],
    [7, 0, 1, 2],
    [3, 4],
    [5, 6],
    [7, 0],
    [1],
    [2],
]


def _register_dve_op(name, spec, subdim=False):
    for op in dve_ops_mod.OPS:
        if op.name == name:
            return op
    row = dve_ops_mod._CUSTOM_DVE_ROW_BASE + len(dve_ops_mod.OPS)
    assert row < 0x20, "custom DVE row overflow"
    dve_ops_mod._SUB_OPCODE_FOR_NAME[name] = row
    rd1 = dve_ops_mod.has_src1(spec)
    shas = {}
    for ver in ("v3", "v4"):
        try:
            s = DveOpSpec(name=name, opcode=row, uops=lower(spec, ver=ver),
                          rd1_en=rd1)
            shas[ver] = s.sha(ver)
        except Exception:
            pass
    op = dve_ops_mod.DveOp(name, spec, subdim=subdim, uops_sha=shas)
    dve_ops_mod.OPS.append(op)
    return op


# out = ((vt + u) <= 0.5) * s1     [s0 = threshold, s1 = hist scale 2^t]
LIF_HIST = _register_dve_op(
    "LIF_HIST_SNN",
    Spec(body=((Src0 + Src1) <= C0) * C1,
         reference=lambda in0, in1, s0, s1: ((in0 + in1) <= s0) * s1))
# out = select((vt + u) <= 0.5, (vt + u) * s1, 0)   [s1 = 0.75]
LIF_VT = _register_dve_op(
    "LIF_VT_SNN",
    Spec(body=select((Src0 + Src1) <= C0, (Src0 + Src1) * C1, Zero),
         reference=lambda in0, in1, s0, s1: np.where(
             (in0 + in1) <= s0, (in0 + in1) * s1, 0.0)))


def _build_banded(w, b, Lin, Lout, Cin, Cout):
    rows_in, rows_out = Lin * Cin, Lout * Cout
    Wd = np.zeros((rows_in, rows_out), np.float32)
    K = w.shape[2]
    for l in range(Lout):
        for k in range(K):
            li = 2 * l + k
            Wd[li * Cin:(li + 1) * Cin, l * Cout:(l + 1) * Cout] = w[:, :, k].T
    bias = np.tile(b, Lout)
    return Wd, bias


def _plan_layers(inp, n_halves):
    """Per layer: tiles with 128-grid-aligned chunks, bias consts.

    Weights stored as `n_halves` fp16 splits; effective weight =
    sum of halves (exact fp16 values).
    """
    wdt = np.float16 if W_NPDT == 'f16' else ml_dtypes.bfloat16
    mats = []
    for i, (Lin, Lout, Cin, Cout) in enumerate(CONV):
        w, b = inp[f'conv{i+1}_w'], inp[f'conv{i+1}_b']
        mats.append(_build_banded(w, b, Lin, Lout, Cin, Cout))
    fw, fb = inp['fc1_w'], inp['fc1_b']
    Wd = np.zeros((216, 256), np.float32)
    for j in range(210):
        l3, co = j // 5, j % 5
        Wd[j, :] = fw[:, co * 42 + l3]
    Wd[210:216, :] = fw[:, 210:216].T
    mats.append((Wd, fb.copy()))
    for i in (2, 3, 4):
        fw, fb = inp[f'fc{i}_w'], inp[f'fc{i}_b']
        mats.append((fw.T.astype(np.float32), fb.copy()))

    layers = []
    for lidx, (Wd, bias) in enumerate(mats):
        rows_in, rows_out = Wd.shape
        ns_rows = np.zeros(rows_in, bool)
        if lidx >= 1:
            ns_rows[:] = True
            if lidx == 3:
                ns_rows[210:216] = False
        Ws = Wd.copy()
        Ws[ns_rows, :] *= -1.0  # stored weight: -W on ns rows
        halves = []
        rem = Ws.astype(np.float64)
        for _ in range(LAYER_HALVES.get(lidx, n_halves)):
            h = rem.astype(np.float32).astype(wdt)
            halves.append(h)
            rem = rem - h.astype(np.float64)
        Weff = np.zeros_like(Ws, np.float64)
        for h in halves:
            Weff += h.astype(np.float64)
        # rowsum of EFFECTIVE stored weights over ns rows, negated back:
        # syn_true = stored @ ns_enc + rowsum ; rowsum = sum_ns(-Weff)
        rowsum = (-Weff * ns_rows[:, None]).sum(axis=0)
        c = bias.astype(np.float64) + rowsum
        tiles = []
        for m0 in range(0, rows_out, 128):
            m1 = min(m0 + 128, rows_out)
            sub = Weff[:, m0:m1]
            nz = np.nonzero(np.any(sub != 0.0, axis=1))[0]
            k0, k1 = int(nz.min()), int(nz.max()) + 1
            chunks = []
            for g in range(k0 // 128, (k1 + 127) // 128):
                a = g * 128
                bnd = min(a + 128, k1, rows_in)
                chunks.append((a, bnd,
                               [np.asarray(h[a:bnd, m0:m1]) for h in halves]))
            tiles.append(dict(m0=m0, m1=m1, chunks=chunks, c=c[m0:m1]))
        layers.append(dict(rows_in=rows_in, rows_out=rows_out, tiles=tiles,
                           G=len(tiles), n_halves=n_halves))
    return layers


def _pack_weights(layers):
    """Pack all chunk halves into one [128, total] fp16 array + beta table."""
    total = 0
    for L in layers:
        for tl in L['tiles']:
            for (a, b_, hs) in tl['chunks']:
                total += hs[0].shape[1] * len(hs)
    wpack = np.zeros((128, total), np.float32)
    off = 0
    for L in layers:
        for tl in L['tiles']:
            tl['offs'] = []
            for (a, b_, hs) in tl['chunks']:
                K, M = hs[0].shape
                hoffs = []
                for h in hs:
                    wpack[:K, off:off + M] = h.astype(np.float32)
                    hoffs.append(off)
                    off += M
                tl['offs'].append(hoffs)
    # beta: per tile column per local tau: c * (2 - 2^-tau)
    ntiles = sum(L['G'] for L in layers)
    btab = np.zeros((128, ntiles * Tc), np.float32)
    ti = 0
    for L in layers:
        for tl in L['tiles']:
            tl['bidx'] = ti
            g = 2.0 - np.power(2.0, -np.arange(Tc, dtype=np.float64))
            btab[:tl['m1'] - tl['m0'], ti * Tc:(ti + 1) * Tc] = (
                tl['c'][:, None] * g[None, :]).astype(np.float32)
            ti += 1
    wdt = np.float16 if W_NPDT == 'f16' else ml_dtypes.bfloat16
    return wpack.astype(wdt), btab


@with_exitstack
def _emit(ctx, tc, layers, wcols, nbt, prm):
    nc = tc.nc
    persist = ctx.enter_context(tc.tile_pool(name="persist", bufs=1))
    scanp = ctx.enter_context(tc.tile_pool(name="scanin", bufs=2))
    psump = ctx.enter_context(tc.tile_pool(name="psum", bufs=1, space="PSUM"))

    WSB_DT = F16 if W_NPDT == 'f16' else BF16
    wsb = persist.tile([128, wcols], WSB_DT, tag="wsb")
    t0_cols = sum(hs[0].shape[1] * len(hs)
                  for (a, b_, hs) in layers[0]['tiles'][0]['chunks'])
    c1_cols = sum(hs[0].shape[1] * len(hs) for tl in layers[0]['tiles']
                  for (a, b_, hs) in tl['chunks'])
    nc.sync.dma_start(wsb[:, :t0_cols], prm['w'][:, :t0_cols])
    bsb = persist.tile([128, nbt], F32, tag="bsb")
    nc.sync.dma_start(bsb[:], prm['bias'][:])
    ident = persist.tile([128, 128], F32, tag="ident")
    nc.sync.dma_start(ident[:], prm['ident'][:])

    hist = persist.tile([128, N_SLOTS, Tb, B], BF16, tag="hist")

    # per-layer contiguous state slices
    goffs = []
    tot = 0
    for L in layers:
        goffs.append(tot)
        tot += L['G']
    usb_all = persist.tile([128, tot * B], F32, tag="usb")
    vtb_all = persist.tile([128, tot * B], F32, tag="vtb")
    usb = [usb_all[:, goffs[i] * B:(goffs[i] + L['G']) * B]
           for i, L in enumerate(layers)]
    vtb = [vtb_all[:, goffs[i] * B:(goffs[i] + L['G']) * B]
           for i, L in enumerate(layers)]
    acc = persist.tile([2, B], F32, tag="acc")
    ns4 = persist.tile([2, B], F32, tag="ns4")

    psum = psump.tile([128, 8 * 512], F32, tag="psum")

    # scratch v tiles for Pool-handled tile ranges
    vscratch = {}
    for li, np_ in POOL_TILES.items():
        if np_ > 0:
            vscr_tile = persist.tile([128, np_ * B], F32,
                                     name=f"vscr{li}", tag=f"vscr{li}")
            vscratch[li] = vscr_tile

    # usb rows beyond each tile's M are read by full-width LIF ops (stay 0:
    # drains write [:M] only). vtb starts 0 (v_0 = u_0). hist slot 12 rows
    # 88.. are read by fc1 chunks and never written.
    nc.vector.memset(usb_all[:], 0.0)
    nc.vector.memset(vtb_all[:], 0.0)
    nc.vector.memset(acc[:], 0.0)
    nc.vector.memset(hist[:, 12], 0.0)

    n_c1 = 3
    nblocks = (T + Tb - 1) // Tb
    for blk in range(nblocks):
        t0 = blk * Tb
        tb = min(Tb, T - t0)
        sc = scanp.tile([128, n_c1, Tb, B], BF16, tag="scan")
        nc.sync.dma_start(sc[:, :, :tb, :], prm['scan2'][:, :, t0:t0 + tb, :])
        if blk == 0:
            nc.sync.dma_start(wsb[:, t0_cols:c1_cols],
                              prm['w'][:, t0_cols:c1_cols])
            nc.sync.dma_start(wsb[:, c1_cols:], prm['w'][:, c1_cols:])
        nc.sync.dma_start(hist[82:88, 12, :tb, :],
                          prm['normal'][:, t0:t0 + tb, :])

        def emit_pkg(li, t, blk=blk, t0=t0, tb=tb, sc=sc):
            L = layers[li]
            G = L['G']
            tiles = L['tiles']
            u_l, vt_l = usb[li], vtb[li]
            s0 = HIST_BASE[li + 1] if li < 6 else None
            t_abs = t0 + t
            tau = t_abs % Tc           # position in the PSUM chain
            banks = [(b + 2 * (t_abs // Tc)) % 8 for b in BANKS[li]]
            sc2 = float(2.0 ** tau)      # rhs/hist scale this step
            sc2m = float(2.0 ** (-tau))  # drain scale

            # --- PE: all chunk matmuls (+carry) of the package, dense ---
            for ti_, tl in enumerate(tiles):
                M = tl['m1'] - tl['m0']
                bk = banks[ti_]
                out_ap = psum[:M, bk * 512:bk * 512 + B]
                first_mm = (tau == 0 and t_abs == 0)
                if tau == 0 and t_abs > 0:
                    # cross-block carry on ACT: bank := Id(0.5*u_prev)
                    # (overwrite; exact). Chunks then accumulate with
                    # start=False on top -- engine-write + matmul-accumulate
                    # mixing is the v6-validated dve_carry pattern.
                    nc.scalar.activation(
                        out_ap, u_l[:M, ti_ * B:(ti_ + 1) * B],
                        AF.Identity, scale=0.5)
                nch = len(tl['chunks'])
                for ci_, ((a, b_, hs), hoffs) in enumerate(
                        zip(tl['chunks'], tl['offs'])):
                    K = b_ - a
                    g_src = a // 128
                    if li == 0:
                        rhs = sc[0:K, g_src, t, :]
                    else:
                        rhs = hist[0:K, HIST_BASE[li] + g_src, t, :]
                    for hi_ in range(len(hs)):
                        st = first_mm and ci_ == 0 and hi_ == 0
                        nc.tensor.matmul(
                            out_ap, wsb[0:K, hoffs[hi_]:hoffs[hi_] + M],
                            rhs,
                            start=st,
                            stop=((tau == Tc - 1 or t_abs == T - 1)
                                  and ci_ == nch - 1
                                  and hi_ == len(hs) - 1),
                            skip_group_check=True)
            # --- ACT: per-tile drains u_true = Id(2^-tau * U + beta) ---
            for ti_, tl in enumerate(tiles):
                M = tl['m1'] - tl['m0']
                bk = banks[ti_]
                out_ap = psum[:M, bk * 512:bk * 512 + B]
                col = tl['bidx'] * Tc + tau
                nc.scalar.activation(
                    u_l[:M, ti_ * B:(ti_ + 1) * B], out_ap,
                    AF.Identity, bias=bsb[:M, col:col + 1], scale=sc2m)

            # ---- LIF: hist = ((vt+u) <= .5)*2^t ; vt' = sel(v<=.5, .75v, 0)
            if li == 6:
                nc.vector._custom_dve(LIF_HIST, out=ns4[:],
                                      in0=vt_l[:2, :], in1=u_l[:2, :],
                                      s0=0.5, s1=1.0)
                nc.vector._custom_dve(LIF_VT, out=vt_l[:2, :],
                                      in0=vt_l[:2, :], in1=u_l[:2, :],
                                      s0=0.5, s1=0.75)
                # acc += ns4 on the idle Pool engine (off the critical
                # chain); final out = 1 - acc/T since ns = 1 - s
                nc.gpsimd.tensor_tensor(acc[:], acc[:], ns4[:], op=OP.add)
                return

            npool = POOL_TILES[li]
            kd = G - npool  # leading tiles on DVE (fused custom ops)
            # DVE tiles: per-tile fused ops (short dependency chains)
            for ti_ in range(kd):
                tl = tiles[ti_]
                M = tl['m1'] - tl['m0']
                sl = slice(ti_ * B, (ti_ + 1) * B)
                if STD_LIF:
                    # v6-style std ops: v overwrites vt in place, then vt
                    # is recomputed from v and hist
                    nc.vector.scalar_tensor_tensor(
                        vt_l[:M, sl], vt_l[:M, sl], 1.0, u_l[:M, sl],
                        op0=OP.mult, op1=OP.add)
                    nc.vector.tensor_scalar(
                        hist[:M, s0 + ti_, t, :], vt_l[:M, sl],
                        0.5, sc2, op0=OP.is_le, op1=OP.mult)
                    nc.vector.scalar_tensor_tensor(
                        vt_l[:M, sl], vt_l[:M, sl], 0.75 * sc2m,
                        hist[:M, s0 + ti_, t, :], op0=OP.mult, op1=OP.mult)
                    continue
                nc.vector._custom_dve(
                    LIF_HIST, out=hist[:M, s0 + ti_, t, :],
                    in0=vt_l[:M, sl], in1=u_l[:M, sl], s0=0.5, s1=sc2)
                nc.vector._custom_dve(
                    LIF_VT, out=vt_l[:M, sl],
                    in0=vt_l[:M, sl], in1=u_l[:M, sl], s0=0.5, s1=0.75)
            # Pool tiles: std 3-op sequence on the contiguous trailing slice
            if npool > 0:
                vscr = vscratch[li]
                psl = slice(kd * B, G * B)
                if li == 2:
                    # conv3 tail tile M=82: write hist [:M] only (slot 12
                    # rows 82.. hold normal spikes / zeros)
                    M = tiles[kd]['m1'] - tiles[kd]['m0']
                    assert npool == 1
                    nc.gpsimd.scalar_tensor_tensor(
                        vscr[:M, :], vt_l[:M, psl], 1.0, u_l[:M, psl],
                        op0=OP.mult, op1=OP.add)
                    nc.gpsimd.tensor_scalar(
                        hist[:M, s0 + kd, t, :], vscr[:M, :512],
                        0.5, sc2, op0=OP.is_le, op1=OP.mult)
                    nc.gpsimd.scalar_tensor_tensor(
                        vt_l[:M, psl], vscr[:M, :512],
                        0.75 * (2.0 ** (-tau)), hist[:M, s0 + kd, t, :],
                        op0=OP.mult, op1=OP.mult)
                else:
                    h3d = hist[:, s0 + kd:s0 + G, t, :]
                    v3d = vscr.rearrange("p (g b) -> p g b", b=B)
                    nc.gpsimd.scalar_tensor_tensor(
                        vscr[:], vt_l[:, psl], 1.0, u_l[:, psl],
                        op0=OP.mult, op1=OP.add)
                    nc.gpsimd.tensor_scalar(
                        h3d, v3d, 0.5, sc2, op0=OP.is_le, op1=OP.mult)
                    nc.gpsimd.scalar_tensor_tensor(
                        vt_l.rearrange("p (g b) -> p g b", b=B)[:, kd:G, :],
                        v3d, 0.75 * (2.0 ** (-tau)), h3d,
                        op0=OP.mult, op1=OP.mult)

        # wavefront emission: conv1 first, then layers 1..6 skewed by 2
        for t in range(tb):
            emit_pkg(0, t)
        rest = sorted((2 * (li - 1) + t, -li, li, t)
                      for li in range(1, 7) for t in range(tb))
        for _, _, li, t in rest:
            emit_pkg(li, t)

    out_sb = persist.tile([2, B], F32, tag="outsb")
    nc.vector.tensor_scalar(out_sb[:], acc[:], -1.0 / T, 1.0,
                            op0=OP.mult, op1=OP.add)
    nc.sync.dma_start(prm['out'][:], out_sb[:])



def build_nc(layers, wcols, nbt):
    nc = bacc_mod.Bacc()
    prm = dict(
        scan2=nc.declare_dram_parameter("scan2", [128, 3 * T * B], BF16,
                                        isOutput=False).rearrange(
                                            "p (s t b) -> p s t b", t=T, b=B),
        normal=nc.declare_dram_parameter("normal", [6, T * B], BF16,
                                         isOutput=False).rearrange(
                                             "l (t b) -> l t b", b=B),
        w=nc.declare_dram_parameter("w", [128, wcols],
                                    F16 if W_NPDT == 'f16' else BF16,
                                    isOutput=False),
        bias=nc.declare_dram_parameter("bias", [128, nbt], F32, isOutput=False),
        ident=nc.declare_dram_parameter("ident", [128, 128], F32,
                                        isOutput=False),
        out=nc.declare_dram_parameter("out", [2, B], F32, isOutput=True),
    )

    with tile.TileContext(nc) as tc:
        _emit(tc, layers, wcols, nbt, prm)
    nc.compile()
    return nc


_NC_CACHE = {}


def kernel(normal_spikes, scan_spikes, batch_size,
           conv1_w, conv1_b, conv2_w, conv2_b, conv3_w, conv3_b,
           fc1_w, fc1_b, fc2_w, fc2_b, fc3_w, fc3_b, fc4_w, fc4_b):
    inp = dict(conv1_w=conv1_w, conv1_b=conv1_b, conv2_w=conv2_w,
               conv2_b=conv2_b, conv3_w=conv3_w, conv3_b=conv3_b,
               fc1_w=fc1_w, fc1_b=fc1_b, fc2_w=fc2_w, fc2_b=fc2_b,
               fc3_w=fc3_w, fc3_b=fc3_b, fc4_w=fc4_w, fc4_b=fc4_b)
    inp = {k: np.asarray(v, np.float32) for k, v in inp.items()}
    layers = _plan_layers(inp, N_HALVES)
    wpack, btab = _pack_weights(layers)
    wcols, nbt = wpack.shape[1], btab.shape[1]

    key = (wcols, nbt, WMODE)
    if key not in _NC_CACHE:
        _NC_CACHE[key] = build_nc(layers, wcols, nbt)
    nc = _NC_CACHE[key]
    kernel._last_nc = nc

    bf = ml_dtypes.bfloat16
    # host prep: time-major feature-major + 2^tau pre-scale (exact in bf16)
    scales = (2.0 ** (np.arange(T) % Tc)).astype(np.float32)  # [T]
    scan_t = np.asarray(scan_spikes)[:, 0].transpose(1, 2, 0)
    scan_t = (scan_t * scales[None, :, None]).astype(bf)
    norm_t = np.asarray(normal_spikes).transpose(1, 2, 0)
    norm_t = (norm_t * scales[None, :, None]).astype(bf)
    ident = np.eye(128, dtype=np.float32) * 0.5
    n_c1 = 3
    scan_rep = np.zeros((128, n_c1, T, B_FULL), bf)
    for g in range(3):
        p = min(128, 360 - g * 128)
        scan_rep[:p, g] = scan_t[g * 128:g * 128 + p]

    in_maps = []
    for c in range(N_CORES):
        sl = slice(c * B, (c + 1) * B)
        in_maps.append(dict(
            scan2=np.ascontiguousarray(
                scan_rep[:, :, :, sl]).reshape(128, n_c1 * T * B),
            normal=np.ascontiguousarray(norm_t[:, :, sl]).reshape(6, T * B),
            w=wpack, bias=btab, ident=ident))
    import time as _time
    t0 = _time.time()
    try:
        res = run_bass_kernel_spmd(nc, in_maps, list(range(N_CORES)))
    except ModuleNotFoundError:
        os.environ["BASS_NEVER_TRACE"] = "1"
        res = run_bass_kernel_spmd(nc, in_maps, list(range(N_CORES)))
    wall1 = _time.time() - t0
    outs = [res.results[c]["out"] for c in range(N_CORES)]

    full = np.concatenate([o.T for o in outs], axis=0).astype(np.float32)
    kernel._last_exec_ns = res.exec_time_ns
    kernel._wall_exec_s = wall1
    it = getattr(res, 'instructions_and_trace', None)
    kernel._last_trace = it[1] if it else None
    return full


# revision 10
# speedup vs baseline: 1.0573x; 1.0323x over previous
"""Trainium2 Bass kernel for nn_ActorNetSpiking — v9 (promoted v8).

Data-parallel over 8 NeuronCores: batch 4096 -> 512 per core.

Changes vs v6 (1513866 ns baseline):
- Weights: fp16 halves (1 or 2) instead of bf16 x3. PE matmul cost is
  keyed on the moving (rhs) dtype; fp16 lhsT x bf16 rhs measured exact
  on HW. 3x (or 1.5x) fewer chunk matmuls.
- LIF elementwise: two custom fused DVE ops replace the v/spike/vt
  passes for DVE-class layers (v never materialized):
    LIF_HIST: hist = ((vt + u) <= 0.5) * 2^t
    LIF_VT:   vt'  = select((vt + u) <= 0.5, (vt + u) * 0.75, 0)
  vt is stored UNSCALED (0.75 * v * ns), so no cross-step scale
  juggling and no t==0 special case (vt memset 0 once).
- Pool-class layers (POOL_LAYERS) run the same math as std 3-op
  sequences on the otherwise-idle GpSimd engine:
    v = (vt mult 1.0) add u; hist = (v is_le .5) * 2^t;
    vt' = (v * 0.75*2^-t) * hist
- u-chain in PSUM, ACT drains with beta, fp32 identity carry: as v6.
"""

import os
import numpy as np
import ml_dtypes

import concourse.bass as bass
import concourse.bacc as bacc_mod
import concourse.tile as tile
from concourse import mybir
from concourse._compat import with_exitstack
from concourse.bass_utils import run_bass_kernel_spmd

import concourse.dve_ops as dve_ops_mod
from concourse.dve_spec import Spec, Src0, Src1, C0, C1, select, Zero, lower
from concourse.dve_uop import DveOpSpec
from concourse.dve_table_gen import dve_ver_for

F32 = mybir.dt.float32
BF16 = mybir.dt.bfloat16
F16 = mybir.dt.float16
AF = mybir.ActivationFunctionType
OP = mybir.AluOpType

N_CORES = 8
B_FULL = 4096
B = B_FULL // N_CORES  # 512 per core
T = 50
Tb = 4  # DMA/hist ring block (hist t-ring depth)
Tc = 4  # PSUM chain length; MUST equal Tb (bank-sharing + hist ring)
WMODE = 'bf16x3'
N_HALVES = 3
LAYER_HALVES = {0: 3, 1: 3, 2: 2, 3: 2, 4: 2, 5: 2, 6: 2}
W_NPDT = 'bf16'
# number of trailing tiles per layer whose LIF runs as std 3-op sequences on
# GpSimd (Pool); leading tiles use the fused custom DVE ops
POOL_TILES = {0: 0, 1: 0, 2: 0, 3: 0, 4: 0, 5: 0}  # GpSimd lacks STT/PSUM
STD_LIF = False

CONV = [  # (Lin, Lout, Cin, Cout)
    (360, 178, 1, 5),
    (178, 87, 5, 5),
    (87, 42, 5, 5),
]

# hist slot base per layer-INPUT (layers 1..6 read hist; layer 0 reads scan)
HIST_BASE = [None, 0, 7, 11, 13, 15, 17]
N_SLOTS = 18
# PSUM bank map per layer (bank index list)
BANKS = [
    [0, 1, 2, 3, 4, 5, 6],
    [7, 0, 1, 2],
    [3, 4],
    [5, 6],
    [7, 0],
    [1],
    [2],
]


def _register_dve_op(name, spec, subdim=False):
    for op in dve_ops_mod.OPS:
        if op.name == name:
            return op
    row = dve_ops_mod._CUSTOM_DVE_ROW_BASE + len(dve_ops_mod.OPS)
    assert row < 0x20, "custom DVE row overflow"
    dve_ops_mod._SUB_OPCODE_FOR_NAME[name] = row
    rd1 = dve_ops_mod.has_src1(spec)
    shas = {}
    for ver in ("v3", "v4"):
        try:
            s = DveOpSpec(name=name, opcode=row, uops=lower(spec, ver=ver),
                          rd1_en=rd1)
            shas[ver] = s.sha(ver)
        except Exception:
            pass
    op = dve_ops_mod.DveOp(name, spec, subdim=subdim, uops_sha=shas)
    dve_ops_mod.OPS.append(op)
    return op


# out = ((vt + u) <= 0.5) * s1     [s0 = threshold, s1 = hist scale 2^t]
LIF_HIST = _register_dve_op(
    "LIF_HIST_SNN",
    Spec(body=((Src0 + Src1) <= C0) * C1,
         reference=lambda in0, in1, s0, s1: ((in0 + in1) <= s0) * s1))
# out = select((vt + u) <= 0.5, (vt + u) * s1, 0)   [s1 = 0.75]
LIF_VT = _register_dve_op(
    "LIF_VT_SNN",
    Spec(body=select((Src0 + Src1) <= C0, (Src0 + Src1) * C1, Zero),
         reference=lambda in0, in1, s0, s1: np.where(
             (in0 + in1) <= s0, (in0 + in1) * s1, 0.0)))


def _build_banded(w, b, Lin, Lout, Cin, Cout):
    rows_in, rows_out = Lin * Cin, Lout * Cout
    Wd = np.zeros((rows_in, rows_out), np.float32)
    K = w.shape[2]
    for l in range(Lout):
        for k in range(K):
            li = 2 * l + k
            Wd[li * Cin:(li + 1) * Cin, l * Cout:(l + 1) * Cout] = w[:, :, k].T
    bias = np.tile(b, Lout)
    return Wd, bias


def _plan_layers(inp, n_halves):
    """Per layer: tiles with 128-grid-aligned chunks, bias consts.

    Weights stored as `n_halves` fp16 splits; effective weight =
    sum of halves (exact fp16 values).
    """
    wdt = np.float16 if W_NPDT == 'f16' else ml_dtypes.bfloat16
    mats = []
    for i, (Lin, Lout, Cin, Cout) in enumerate(CONV):
        w, b = inp[f'conv{i+1}_w'], inp[f'conv{i+1}_b']
        mats.append(_build_banded(w, b, Lin, Lout, Cin, Cout))
    fw, fb = inp['fc1_w'], inp['fc1_b']
    Wd = np.zeros((216, 256), np.float32)
    for j in range(210):
        l3, co = j // 5, j % 5
        Wd[j, :] = fw[:, co * 42 + l3]
    Wd[210:216, :] = fw[:, 210:216].T
    mats.append((Wd, fb.copy()))
    for i in (2, 3, 4):
        fw, fb = inp[f'fc{i}_w'], inp[f'fc{i}_b']
        mats.append((fw.T.astype(np.float32), fb.copy()))

    layers = []
    for lidx, (Wd, bias) in enumerate(mats):
        rows_in, rows_out = Wd.shape
        ns_rows = np.zeros(rows_in, bool)
        if lidx >= 1:
            ns_rows[:] = True
            if lidx == 3:
                ns_rows[210:216] = False
        Ws = Wd.copy()
        Ws[ns_rows, :] *= -1.0  # stored weight: -W on ns rows
        halves = []
        rem = Ws.astype(np.float64)
        for _ in range(LAYER_HALVES.get(lidx, n_halves)):
            h = rem.astype(np.float32).astype(wdt)
            halves.append(h)
            rem = rem - h.astype(np.float64)
        Weff = np.zeros_like(Ws, np.float64)
        for h in halves:
            Weff += h.astype(np.float64)
        # rowsum of EFFECTIVE stored weights over ns rows, negated back:
        # syn_true = stored @ ns_enc + rowsum ; rowsum = sum_ns(-Weff)
        rowsum = (-Weff * ns_rows[:, None]).sum(axis=0)
        c = bias.astype(np.float64) + rowsum
        tiles = []
        for m0 in range(0, rows_out, 128):
            m1 = min(m0 + 128, rows_out)
            sub = Weff[:, m0:m1]
            nz = np.nonzero(np.any(sub != 0.0, axis=1))[0]
            k0, k1 = int(nz.min()), int(nz.max()) + 1
            chunks = []
            for g in range(k0 // 128, (k1 + 127) // 128):
                a = g * 128
                bnd = min(a + 128, k1, rows_in)
                chunks.append((a, bnd,
                               [np.asarray(h[a:bnd, m0:m1]) for h in halves]))
            tiles.append(dict(m0=m0, m1=m1, chunks=chunks, c=c[m0:m1]))
        layers.append(dict(rows_in=rows_in, rows_out=rows_out, tiles=tiles,
                           G=len(tiles), n_halves=n_halves))
    return layers


def _pack_weights(layers):
    """Pack all chunk halves into one [128, total] fp16 array + beta table."""
    total = 0
    for L in layers:
        for tl in L['tiles']:
            for (a, b_, hs) in tl['chunks']:
                total += hs[0].shape[1] * len(hs)
    wpack = np.zeros((128, total), np.float32)
    off = 0
    for L in layers:
        for tl in L['tiles']:
            tl['offs'] = []
            for (a, b_, hs) in tl['chunks']:
                K, M = hs[0].shape
                hoffs = []
                for h in hs:
                    wpack[:K, off:off + M] = h.astype(np.float32)
                    hoffs.append(off)
                    off += M
                tl['offs'].append(hoffs)
    # beta: per tile column per local tau: c * (2 - 2^-tau)
    ntiles = sum(L['G'] for L in layers)
    btab = np.zeros((128, ntiles * Tc), np.float32)
    ti = 0
    for L in layers:
        for tl in L['tiles']:
            tl['bidx'] = ti
            g = 2.0 - np.power(2.0, -np.arange(Tc, dtype=np.float64))
            btab[:tl['m1'] - tl['m0'], ti * Tc:(ti + 1) * Tc] = (
                tl['c'][:, None] * g[None, :]).astype(np.float32)
            ti += 1
    wdt = np.float16 if W_NPDT == 'f16' else ml_dtypes.bfloat16
    return wpack.astype(wdt), btab


@with_exitstack
def _emit(ctx, tc, layers, wcols, nbt, prm):
    nc = tc.nc
    persist = ctx.enter_context(tc.tile_pool(name="persist", bufs=1))
    scanp = ctx.enter_context(tc.tile_pool(name="scanin", bufs=2))
    psump = ctx.enter_context(tc.tile_pool(name="psum", bufs=1, space="PSUM"))

    WSB_DT = F16 if W_NPDT == 'f16' else BF16
    wsb = persist.tile([128, wcols], WSB_DT, tag="wsb")
    t0_cols = sum(hs[0].shape[1] * len(hs)
                  for (a, b_, hs) in layers[0]['tiles'][0]['chunks'])
    c1_cols = sum(hs[0].shape[1] * len(hs) for tl in layers[0]['tiles']
                  for (a, b_, hs) in tl['chunks'])
    nc.sync.dma_start(wsb[:, :t0_cols], prm['w'][:, :t0_cols])
    bsb = persist.tile([128, nbt], F32, tag="bsb")
    nc.sync.dma_start(bsb[:], prm['bias'][:])
    ident = persist.tile([128, 128], F32, tag="ident")
    nc.sync.dma_start(ident[:], prm['ident'][:])

    hist = persist.tile([128, N_SLOTS, Tb, B], BF16, tag="hist")

    # per-layer contiguous state slices
    goffs = []
    tot = 0
    for L in layers:
        goffs.append(tot)
        tot += L['G']
    usb_all = persist.tile([128, tot * B], F32, tag="usb")
    vtb_all = persist.tile([128, tot * B], F32, tag="vtb")
    usb = [usb_all[:, goffs[i] * B:(goffs[i] + L['G']) * B]
           for i, L in enumerate(layers)]
    vtb = [vtb_all[:, goffs[i] * B:(goffs[i] + L['G']) * B]
           for i, L in enumerate(layers)]
    acc = persist.tile([2, B], F32, tag="acc")
    ns4 = persist.tile([2, B], F32, tag="ns4")
    v4scr = persist.tile([2, B], F32, tag="v4scr")
    g4scr = persist.tile([2, B], F32, tag="g4scr")

    psum = psump.tile([128, 8 * 512], F32, tag="psum")

    # scratch v tiles for Pool-handled tile ranges
    vscratch = {}
    for li, np_ in POOL_TILES.items():
        if np_ > 0:
            vscr_tile = persist.tile([128, np_ * B], F32,
                                     name=f"vscr{li}", tag=f"vscr{li}")
            vscratch[li] = vscr_tile

    # usb rows beyond each tile's M are read by full-width LIF ops (stay 0:
    # drains write [:M] only). vtb starts 0 (v_0 = u_0). hist slot 12 rows
    # 88.. are read by fc1 chunks and never written.
    nc.vector.memset(usb_all[:], 0.0)
    nc.vector.memset(vtb_all[:], 0.0)
    nc.vector.memset(acc[:], 0.0)
    nc.vector.memset(hist[:, 12], 0.0)

    n_c1 = 3
    nblocks = (T + Tb - 1) // Tb
    for blk in range(nblocks):
        t0 = blk * Tb
        tb = min(Tb, T - t0)
        sc = scanp.tile([128, n_c1, Tb, B], BF16, tag="scan")
        nc.sync.dma_start(sc[:, :, :tb, :], prm['scan2'][:, :, t0:t0 + tb, :])
        if blk == 0:
            nc.sync.dma_start(wsb[:, t0_cols:c1_cols],
                              prm['w'][:, t0_cols:c1_cols])
            nc.sync.dma_start(wsb[:, c1_cols:], prm['w'][:, c1_cols:])
        nc.sync.dma_start(hist[82:88, 12, :tb, :],
                          prm['normal'][:, t0:t0 + tb, :])

        def emit_pkg(li, t, blk=blk, t0=t0, tb=tb, sc=sc):
            L = layers[li]
            G = L['G']
            tiles = L['tiles']
            u_l, vt_l = usb[li], vtb[li]
            s0 = HIST_BASE[li + 1] if li < 6 else None
            t_abs = t0 + t
            tau = t_abs % Tc           # position in the PSUM chain
            banks = [(b + 2 * (t_abs // Tc)) % 8 for b in BANKS[li]]
            sc2 = float(2.0 ** tau)      # rhs/hist scale this step
            sc2m = float(2.0 ** (-tau))  # drain scale

            # --- PE: all chunk matmuls (+carry) of the package, dense ---
            for ti_, tl in enumerate(tiles):
                M = tl['m1'] - tl['m0']
                bk = banks[ti_]
                out_ap = psum[:M, bk * 512:bk * 512 + B]
                first_mm = (tau == 0 and t_abs == 0)
                if tau == 0 and t_abs > 0:
                    # cross-block carry on ACT: bank := Id(0.5*u_prev)
                    # (overwrite; exact). Chunks then accumulate with
                    # start=False on top -- engine-write + matmul-accumulate
                    # mixing is the v6-validated dve_carry pattern.
                    nc.scalar.activation(
                        out_ap, u_l[:M, ti_ * B:(ti_ + 1) * B],
                        AF.Identity, scale=0.5)
                nch = len(tl['chunks'])
                for ci_, ((a, b_, hs), hoffs) in enumerate(
                        zip(tl['chunks'], tl['offs'])):
                    K = b_ - a
                    g_src = a // 128
                    if li == 0:
                        rhs = sc[0:K, g_src, t, :]
                    else:
                        rhs = hist[0:K, HIST_BASE[li] + g_src, t, :]
                    for hi_ in range(len(hs)):
                        st = first_mm and ci_ == 0 and hi_ == 0
                        nc.tensor.matmul(
                            out_ap, wsb[0:K, hoffs[hi_]:hoffs[hi_] + M],
                            rhs,
                            start=st,
                            stop=((tau == Tc - 1 or t_abs == T - 1)
                                  and ci_ == nch - 1
                                  and hi_ == len(hs) - 1),
                            skip_group_check=True)
            # --- ACT: per-tile drains u_true = Id(2^-tau * U + beta) ---
            for ti_, tl in enumerate(tiles):
                M = tl['m1'] - tl['m0']
                bk = banks[ti_]
                out_ap = psum[:M, bk * 512:bk * 512 + B]
                col = tl['bidx'] * Tc + tau
                nc.scalar.activation(
                    u_l[:M, ti_ * B:(ti_ + 1) * B], out_ap,
                    AF.Identity, bias=bsb[:M, col:col + 1], scale=sc2m)

            # ---- LIF: hist = ((vt+u) <= .5)*2^t ; vt' = sel(v<=.5, .75v, 0)
            if li == 6:
                # fc4 LIF entirely on Pool (TT/TS only; bit-identical):
                # v4 = vt+u; ns4 = (v4<=.5); g4 = ns4*0.75; vt4 = v4*g4
                nc.gpsimd.tensor_tensor(v4scr[:], vt_l[:2, :], u_l[:2, :],
                                        op=OP.add)
                nc.gpsimd.tensor_scalar(ns4[:], v4scr[:], 0.5, 1.0,
                                        op0=OP.is_le, op1=OP.mult)
                nc.gpsimd.tensor_scalar(g4scr[:], v4scr[:], 0.5, 0.75,
                                        op0=OP.is_le, op1=OP.mult)
                nc.gpsimd.tensor_tensor(vt_l[:2, :], v4scr[:], g4scr[:],
                                        op=OP.mult)
                # acc += ns4 on the idle Pool engine (off the critical
                # chain); final out = 1 - acc/T since ns = 1 - s
                nc.gpsimd.tensor_tensor(acc[:], acc[:], ns4[:], op=OP.add)
                return

            npool = POOL_TILES[li]
            kd = G - npool  # leading tiles on DVE (fused custom ops)
            # DVE tiles: per-tile fused ops (short dependency chains)
            for ti_ in range(kd):
                tl = tiles[ti_]
                M = tl['m1'] - tl['m0']
                sl = slice(ti_ * B, (ti_ + 1) * B)
                if STD_LIF:
                    # v6-style std ops: v overwrites vt in place, then vt
                    # is recomputed from v and hist
                    nc.vector.scalar_tensor_tensor(
                        vt_l[:M, sl], vt_l[:M, sl], 1.0, u_l[:M, sl],
                        op0=OP.mult, op1=OP.add)
                    nc.vector.tensor_scalar(
                        hist[:M, s0 + ti_, t, :], vt_l[:M, sl],
                        0.5, sc2, op0=OP.is_le, op1=OP.mult)
                    nc.vector.scalar_tensor_tensor(
                        vt_l[:M, sl], vt_l[:M, sl], 0.75 * sc2m,
                        hist[:M, s0 + ti_, t, :], op0=OP.mult, op1=OP.mult)
                    continue
                nc.vector._custom_dve(
                    LIF_HIST, out=hist[:M, s0 + ti_, t, :],
                    in0=vt_l[:M, sl], in1=u_l[:M, sl], s0=0.5, s1=sc2)
                nc.vector._custom_dve(
                    LIF_VT, out=vt_l[:M, sl],
                    in0=vt_l[:M, sl], in1=u_l[:M, sl], s0=0.5, s1=0.75)
            # Pool tiles: std 3-op sequence on the contiguous trailing slice
            if npool > 0:
                vscr = vscratch[li]
                psl = slice(kd * B, G * B)
                if li == 2:
                    # conv3 tail tile M=82: write hist [:M] only (slot 12
                    # rows 82.. hold normal spikes / zeros)
                    M = tiles[kd]['m1'] - tiles[kd]['m0']
                    assert npool == 1
                    nc.gpsimd.scalar_tensor_tensor(
                        vscr[:M, :], vt_l[:M, psl], 1.0, u_l[:M, psl],
                        op0=OP.mult, op1=OP.add)
                    nc.gpsimd.tensor_scalar(
                        hist[:M, s0 + kd, t, :], vscr[:M, :512],
                        0.5, sc2, op0=OP.is_le, op1=OP.mult)
                    nc.gpsimd.scalar_tensor_tensor(
                        vt_l[:M, psl], vscr[:M, :512],
                        0.75 * (2.0 ** (-tau)), hist[:M, s0 + kd, t, :],
                        op0=OP.mult, op1=OP.mult)
                else:
                    h3d = hist[:, s0 + kd:s0 + G, t, :]
                    v3d = vscr.rearrange("p (g b) -> p g b", b=B)
                    nc.gpsimd.scalar_tensor_tensor(
                        vscr[:], vt_l[:, psl], 1.0, u_l[:, psl],
                        op0=OP.mult, op1=OP.add)
                    nc.gpsimd.tensor_scalar(
                        h3d, v3d, 0.5, sc2, op0=OP.is_le, op1=OP.mult)
                    nc.gpsimd.scalar_tensor_tensor(
                        vt_l.rearrange("p (g b) -> p g b", b=B)[:, kd:G, :],
                        v3d, 0.75 * (2.0 ** (-tau)), h3d,
                        op0=OP.mult, op1=OP.mult)

        # wavefront emission: conv1 first, then layers 1..6 skewed by 2
        for t in range(tb):
            emit_pkg(0, t)
        rest = sorted((2 * (li - 1) + t, -li, li, t)
                      for li in range(1, 7) for t in range(tb))
        for _, _, li, t in rest:
            emit_pkg(li, t)

    out_sb = persist.tile([2, B], F32, tag="outsb")
    nc.vector.tensor_scalar(out_sb[:], acc[:], -1.0 / T, 1.0,
                            op0=OP.mult, op1=OP.add)
    nc.sync.dma_start(prm['out'][:], out_sb[:])



def build_nc(layers, wcols, nbt):
    nc = bacc_mod.Bacc()
    prm = dict(
        scan2=nc.declare_dram_parameter("scan2", [128, 3 * T * B], BF16,
                                        isOutput=False).rearrange(
                                            "p (s t b) -> p s t b", t=T, b=B),
        normal=nc.declare_dram_parameter("normal", [6, T * B], BF16,
                                         isOutput=False).rearrange(
                                             "l (t b) -> l t b", b=B),
        w=nc.declare_dram_parameter("w", [128, wcols],
                                    F16 if W_NPDT == 'f16' else BF16,
                                    isOutput=False),
        bias=nc.declare_dram_parameter("bias", [128, nbt], F32, isOutput=False),
        ident=nc.declare_dram_parameter("ident", [128, 128], F32,
                                        isOutput=False),
        out=nc.declare_dram_parameter("out", [2, B], F32, isOutput=True),
    )

    with tile.TileContext(nc) as tc:
        _emit(tc, layers, wcols, nbt, prm)
    nc.compile()
    return nc


_NC_CACHE = {}


def kernel(normal_spikes, scan_spikes, batch_size,
           conv1_w, conv1_b, conv2_w, conv2_b, conv3_w, conv3_b,
           fc1_w, fc1_b, fc2_w, fc2_b, fc3_w, fc3_b, fc4_w, fc4_b):
    inp = dict(conv1_w=conv1_w, conv1_b=conv1_b, conv2_w=conv2_w,
               conv2_b=conv2_b, conv3_w=conv3_w, conv3_b=conv3_b,
               fc1_w=fc1_w, fc1_b=fc1_b, fc2_w=fc2_w, fc2_b=fc2_b,
               fc3_w=fc3_w, fc3_b=fc3_b, fc4_w=fc4_w, fc4_b=fc4_b)
    inp = {k: np.asarray(v, np.float32) for k, v in inp.items()}
    layers = _plan_layers(inp, N_HALVES)
    wpack, btab = _pack_weights(layers)
    wcols, nbt = wpack.shape[1], btab.shape[1]

    key = (wcols, nbt, WMODE)
    if key not in _NC_CACHE:
        _NC_CACHE[key] = build_nc(layers, wcols, nbt)
    nc = _NC_CACHE[key]
    kernel._last_nc = nc

    bf = ml_dtypes.bfloat16
    # host prep: time-major feature-major + 2^tau pre-scale (exact in bf16)
    scales = (2.0 ** (np.arange(T) % Tc)).astype(np.float32)  # [T]
    scan_t = np.asarray(scan_spikes)[:, 0].transpose(1, 2, 0)
    scan_t = (scan_t * scales[None, :, None]).astype(bf)
    norm_t = np.asarray(normal_spikes).transpose(1, 2, 0)
    norm_t = (norm_t * scales[None, :, None]).astype(bf)
    ident = np.eye(128, dtype=np.float32) * 0.5
    n_c1 = 3
    scan_rep = np.zeros((128, n_c1, T, B_FULL), bf)
    for g in range(3):
        p = min(128, 360 - g * 128)
        scan_rep[:p, g] = scan_t[g * 128:g * 128 + p]

    in_maps = []
    for c in range(N_CORES):
        sl = slice(c * B, (c + 1) * B)
        in_maps.append(dict(
            scan2=np.ascontiguousarray(
                scan_rep[:, :, :, sl]).reshape(128, n_c1 * T * B),
            normal=np.ascontiguousarray(norm_t[:, :, sl]).reshape(6, T * B),
            w=wpack, bias=btab, ident=ident))
    import time as _time
    t0 = _time.time()
    try:
        res = run_bass_kernel_spmd(nc, in_maps, list(range(N_CORES)))
    except ModuleNotFoundError:
        os.environ["BASS_NEVER_TRACE"] = "1"
        res = run_bass_kernel_spmd(nc, in_maps, list(range(N_CORES)))
    wall1 = _time.time() - t0
    outs = [res.results[c]["out"] for c in range(N_CORES)]

    full = np.concatenate([o.T for o in outs], axis=0).astype(np.float32)
    kernel._last_exec_ns = res.exec_time_ns
    kernel._wall_exec_s = wall1
    it = getattr(res, 'instructions_and_trace', None)
    kernel._last_trace = it[1] if it else None
    return full
